# revision 13
# baseline (speedup 1.0000x reference)
"""Trainium2 Bass kernel for nn_MHA_48120813584614 (dual cross-attention MHA).

v2 strategy (head/tensor parallel over 8 cores, merged-weight formulation):
  - Core c owns head c for BOTH attention directions:
      pair 0 ("i"): metadata queries attend image keys/values  -> out_i
      pair 1 ("m"): image queries attend metadata keys/values -> out_m
  - Host precomputes merged weights per head, eliminating two of the four
    projections (K and V are never materialized):
      G  = Wq^T Wk * WSG      (scores  S = Xq G Xkv^T + kb,  kb = Xkv gb,
      gb = Wk^T bq * WSG       with q-bias folded into the exp bias; the
                               k-bias and const terms cancel in softmax)
      Gl = Wv^T Wl^T * WSL    (head output partial = Gl^T (Xkv^T E); the
                               v-bias term becomes Wl bv after softmax
                               normalization, folded into the host resid)
  - Device per pair:  A = G^T XqT           [proj, PE]
      ST[n,m] = xt_kv^T A  (keys on partitions, 2 PSUM banks per n-tile
                covering two query chunks mc/mc+1 of the same n-tile)
      E = exp(SCALE*ST + bias_nt), bias_nt = EXP_BIAS + SCALE*kbT[:,nt]
                (one ACT instr per 2-bank group; kbT computed by tiny
                free=1 matmuls so the bias is per-partition correct)
      colsum = onesWSL^T E  (softmax denominator on PE, all rows equal)
      OT' = x_nat^T E       (PV directly against the raw token-major input)
      partial = Gl^T OT' ; res = partial * (1/colsum)  [deferred softmax]
  - Row-parallel output linear: per-mc ReduceScatter chunks as in v1; each
    core adds its 64-row resid slice (which now carries b_lin + residual +
    sum_h Wl_h bv_h) and returns the transposed shard.
  - All matmuls fp8e4 DoubleRow (0.5 cy/col). Per-engine balance per rep:
    PE ~181k cy (~76us), ACT = exp only (64 two-bank instrs ~64us), DVE =
    evacuations (~58us), Pool = epilogue adds. The emission is a 5-stage
    software pipeline; stage s interleaves scores(s+1) fills with PV(s)/
    colsum(s)/linear(s-1) so the in-order PE never waits on exp drains and
    stays in its 2.4GHz boost p-state.
"""

import sys

sys.path.insert(0, "/opt/trn_rl_repo")

import math

import ml_dtypes
import numpy as np

import concourse.bass as bass
import concourse.mybir as mybir
import concourse.tile as tile
from concourse import bacc
from concourse.bass_utils import run_bass_kernel_spmd

H = 8
D = 512
N = 2048
NCORES = 8
P = 128
MC = 512  # query chunk (PSUM bank free size)
NMC = N // MC  # 4
NMC2 = NMC // 2  # 2 (two-bank query super-chunks)
ET = D // P  # 4 feature tiles
NT = N // P  # 16 key tiles
WSG = 8.0  # host prescale on G/gb (lifts fp8 subnormals)
WSL = 8.0  # host prescale on Gl (cancelled via colsum)
SCALE = 1.0 / math.sqrt(D) / WSG
# E is exp(logits)-ln4 so the PV accumulation fits fp8e4 range; the factor
# cancels between OT' and the colsum.
EXP_BIAS = -math.log(4.0)

bf16 = mybir.dt.bfloat16
f32 = mybir.dt.float32
fp8 = mybir.dt.float8e4

AF = mybir.ActivationFunctionType
DR = mybir.MatmulPerfMode.DoubleRow


def _build(reps=1, single=False, no_cc=False):
    ndev = 1 if single else NCORES
    nc = bacc.Bacc("TRN2", target_bir_lowering=False, debug=False, num_devices=ndev)

    def din(name, shape, dtype):
        return nc.dram_tensor(name, shape, dtype, kind="ExternalInput").ap()

    xt_d = [din("xt_i", [D, N], fp8), din("xt_m", [D, N], fp8)]
    xn_d = [din("xn_i", [N, D], fp8), din("xn_m", [N, D], fp8)]
    g_d = [din(f"g{p}", [D, D], fp8) for p in range(2)]
    gl_d = [din(f"gl{p}", [D, D], fp8) for p in range(2)]
    gb_d = [din(f"gb{p}", [D, 1], fp8) for p in range(2)]
    resid_d = [din("resid_i", [64, N], f32), din("resid_m", [64, N], f32)]
    out_d = [
        nc.dram_tensor("out_i", [64, N], f32, kind="ExternalOutput").ap(),
        nc.dram_tensor("out_m", [64, N], f32, kind="ExternalOutput").ap(),
    ]

    with tile.TileContext(nc) as tc:
        with (
            tc.tile_pool(name="const", bufs=2) as cpool,
            tc.tile_pool(name="xin", bufs=1) as xpool,
            tc.tile_pool(name="w", bufs=2) as wpool,
            tc.tile_pool(name="a", bufs=1) as apool,
            tc.tile_pool(name="e2", bufs=2) as epool,
            tc.tile_pool(name="ot", bufs=2) as otpool,
            tc.tile_pool(name="rb", bufs=2) as rbpool,
            tc.tile_pool(name="res", bufs=2) as respool,
            tc.tile_pool(name="small", bufs=2) as spool,
            tc.tile_pool(name="post", bufs=2) as ppool,
            tc.tile_pool(name="pss", bufs=2, space="PSUM") as pss,
            tc.tile_pool(name="pw", bufs=2, space="PSUM") as pw,
            tc.tile_pool(name="dram", bufs=1, space="DRAM") as dr,
        ):
          for _rep in range(reps):
            # ---- input DMAs (Pool hw queue: keeps them off the ACT engine,
            # which the exp stream saturates, and off the SP store queue) ----
            # order tuned so A0 (g0 x xt_m) can start earliest.
            gs, gls, gbs = [None, None], [None, None], [None, None]
            gs[0] = wpool.tile([P, ET, D], fp8, tag="g0", name="g0")
            nc.gpsimd.dma_start(gs[0][:], g_d[0].rearrange("(t p) e -> p t e", p=P))
            xts = [xpool.tile([P, ET, N], fp8, tag=f"xt{i}", name=f"xt{i}")
                   for i in range(2)]
            xns = [xpool.tile([P, NT, D], fp8, tag=f"xn{i}", name=f"xn{i}")
                   for i in range(2)]
            nc.gpsimd.dma_start(xts[1][:], xt_d[1].rearrange("(t p) n -> p t n", p=P))
            nc.gpsimd.dma_start(xts[0][:], xt_d[0].rearrange("(t p) n -> p t n", p=P))
            gbs[0] = wpool.tile([P, ET, 1], fp8, tag="gb0", name="gb0")
            nc.gpsimd.dma_start(gbs[0][:], gb_d[0].rearrange("(t p) o -> p t o", p=P))
            nc.gpsimd.dma_start(xns[0][:], xn_d[0].rearrange("(t p) d -> p t d", p=P))
            gls[0] = wpool.tile([P, ET, D], fp8, tag="gl0", name="gl0")
            nc.gpsimd.dma_start(gls[0][:], gl_d[0].rearrange("(t p) e -> p t e", p=P))
            gs[1] = wpool.tile([P, ET, D], fp8, tag="g1", name="g1")
            nc.gpsimd.dma_start(gs[1][:], g_d[1].rearrange("(t p) e -> p t e", p=P))
            gbs[1] = wpool.tile([P, ET, 1], fp8, tag="gb1", name="gb1")
            nc.gpsimd.dma_start(gbs[1][:], gb_d[1].rearrange("(t p) o -> p t o", p=P))
            nc.gpsimd.dma_start(xns[1][:], xn_d[1].rearrange("(t p) d -> p t d", p=P))
            gls[1] = wpool.tile([P, ET, D], fp8, tag="gl1", name="gl1")
            nc.gpsimd.dma_start(gls[1][:], gl_d[1].rearrange("(t p) e -> p t e", p=P))

            # 128 identical columns of WSL (dual-fp8 LdWeights rejects
            # narrow stationaries); every colsum PSUM row is identical so
            # the reciprocal needs no partition broadcast.
            ones8 = cpool.tile([P, 2, P], fp8, tag="ones8")
            nc.any.memset(ones8[:], WSL)
            resid_sb = []
            for i in range(2):
                rt = cpool.tile([64, N], f32, tag=f"resid{i}")
                nc.gpsimd.dma_start(rt[:], resid_d[i][:])
                resid_sb.append(rt)

            a_t = [apool.tile([P, ET, N], fp8, tag=f"a{p}", name=f"a{p}")
                   for p in range(2)]
            bpair = [spool.tile([P, NT], f32, tag=f"bp{p}", name=f"bp{p}")
                     for p in range(2)]

            rs_in = [[dr.tile([D, MC], bf16, tag=f"rsin{p}{mc}",
                              name=f"rsin{p}{mc}") for mc in range(NMC)]
                     for p in range(2)]
            rs_out = [[dr.tile([64, MC], bf16, tag=f"rsout{p}{mc}",
                               name=f"rsout{p}{mc}") for mc in range(NMC)]
                      for p in range(2)]

            # ---- unit emitters ----
            def evac_copy(dst, src, engine):
                if engine == "act":
                    nc.scalar.activation(dst, src, AF.Identity)
                else:
                    nc.vector.tensor_copy(dst, src)

            def a_units(p, alt=False):
                """A = G^T XqT: closures per (query-chunk mc, e'-tile pair).
                alt=True alternates the evacuation between ACT and DVE (used
                in the prologue where ACT has no exp stream yet)."""
                xq = xts[1 - p]
                units = []
                for mc in range(NMC):
                    for ebp in range(ET // 2):
                        def u(mc=mc, ebp=ebp):
                            psa = pss.tile([P, 2, MC], f32, tag="pss", name="psa")
                            for half in range(2):
                                eb = 2 * ebp + half
                                for dt_ in range(ET // 2):
                                    nc.tensor.matmul(
                                        psa[:, half:half + 1, :],
                                        gs[p][:, 2 * dt_:2 * dt_ + 2,
                                              eb * P:(eb + 1) * P],
                                        xq[:, 2 * dt_:2 * dt_ + 2,
                                           mc * MC:(mc + 1) * MC],
                                        start=(dt_ == 0),
                                        stop=(dt_ == ET // 2 - 1),
                                        perf_mode=DR,
                                    )
                            eng = "act" if alt and (mc * 2 + ebp) % 2 else "dve"
                            evac_copy(
                                a_t[p][:, 2 * ebp:2 * ebp + 2,
                                       mc * MC:(mc + 1) * MC],
                                psa[:], eng,
                            )
                        units.append(u)
                return units

            def kb_unit(p):
                """kbT[n,1] per n-tile via free=1 matmuls; then the exp bias
                tile bias_nt = EXP_BIAS + SCALE*kbT in one DVE op."""
                def u():
                    kbps = pw.tile([P, 2, MC], f32, tag="pw", name="kbps")
                    for nt in range(NT):
                        for dt_ in range(ET // 2):
                            nc.tensor.matmul(
                                kbps[:, 0:1, nt:nt + 1],
                                xts[p][:, 2 * dt_:2 * dt_ + 2,
                                       nt * P:(nt + 1) * P],
                                gbs[p][:, 2 * dt_:2 * dt_ + 2, 0:1],
                                start=(dt_ == 0),
                                stop=(dt_ == ET // 2 - 1),
                                perf_mode=DR,
                            )
                    nc.vector.tensor_scalar(
                        bpair[p][:], kbps[:, 0, 0:NT], SCALE, EXP_BIAS,
                        mybir.AluOpType.mult, mybir.AluOpType.add,
                    )
                return u

            def scores_units(p, mc2):
                """16 closures; each fills a 2-bank score group (same n-tile,
                query chunks 2*mc2 / 2*mc2+1) and exps it into e2."""
                e2 = epool.tile([P, NT, 2, MC], fp8, tag="e2", name="e2")
                units = []
                for nt in range(NT):
                    def u(nt=nt):
                        pst = pss.tile([P, 2, MC], f32, tag="pss", name="pst")
                        for half in range(2):
                            mc = 2 * mc2 + half
                            for dt_ in range(ET // 2):
                                nc.tensor.matmul(
                                    pst[:, half:half + 1, :],
                                    xts[p][:, 2 * dt_:2 * dt_ + 2,
                                           nt * P:(nt + 1) * P],
                                    a_t[p][:, 2 * dt_:2 * dt_ + 2,
                                           mc * MC:(mc + 1) * MC],
                                    start=(dt_ == 0),
                                    stop=(dt_ == ET // 2 - 1),
                                    perf_mode=DR,
                                )
                        nc.scalar.activation(
                            e2[:, nt:nt + 1, :, :], pst[:], AF.Exp,
                            scale=SCALE, bias=bpair[p][:, nt:nt + 1],
                        )
                    units.append(u)
                return e2, units

            # deferred epilogue chunks (Pool + DMA only, no PE)
            po_queue = []

            def po_chunk(p, mc):
                def emit():
                    po_bf = ppool.tile([64, MC], bf16, tag="pobf", name="pobf")
                    src = (rs_out[p][mc][:] if not (single or no_cc)
                           else rs_in[p][mc][0:64, :])
                    nc.sync.dma_start(po_bf[:], src)
                    po = ppool.tile([64, MC], f32, tag="po", name="po")
                    nc.gpsimd.tensor_tensor(
                        po[:], po_bf[:], resid_sb[p][:, mc * MC:(mc + 1) * MC],
                        mybir.AluOpType.add,
                    )
                    nc.sync.dma_start(out_d[p][:, mc * MC:(mc + 1) * MC], po[:])
                return emit

            def pv_cs_units(p, mc2, e2, ot_eng="dve"):
                """PV + colsum for super-chunk mc2 (consumes completed e2);
                produces otb (for the next stage's linear) and rb2."""
                otb = otpool.tile([P, ET, 2, MC], fp8, tag="ot", name="otb")
                rb2 = rbpool.tile([P, 2, MC], f32, tag="rb", name="rb2")
                units = []
                for dto in range(ET):
                    def u(dto=dto):
                        pso = pw.tile([P, 2, MC], f32, tag="pw", name="pso")
                        for half in range(2):
                            for j in range(NT // 2):
                                nc.tensor.matmul(
                                    pso[:, half:half + 1, :],
                                    xns[p][:, 2 * j:2 * j + 2,
                                           dto * P:(dto + 1) * P],
                                    e2[:, 2 * j:2 * j + 2, half:half + 1, :],
                                    start=(j == 0),
                                    stop=(j == NT // 2 - 1),
                                    perf_mode=DR,
                                )
                        evac_copy(otb[:, dto:dto + 1, :, :], pso[:], ot_eng)
                    units.append(u)

                def u_cs():
                    cs2 = pw.tile([P, 2, MC], f32, tag="pw", name="cs2")
                    for half in range(2):
                        for j in range(NT // 2):
                            nc.tensor.matmul(
                                cs2[:, half:half + 1, :],
                                ones8[:],
                                e2[:, 2 * j:2 * j + 2, half:half + 1, :],
                                start=(j == 0),
                                stop=(j == NT // 2 - 1),
                                perf_mode=DR,
                            )
                    nc.vector.reciprocal(rb2[:], cs2[:])
                units.append(u_cs)
                return (otb, rb2), units

            def linear_units(p, mc2, otb_rb):
                """Output-linear partials + deferred softmax normalization +
                per-mc ReduceScatter chunks for super-chunk mc2."""
                otb, rb2 = otb_rb
                units = []
                for ob in range(ET):
                    def u(ob=ob):
                        psl = pw.tile([P, 2, MC], f32, tag="pw", name="psl")
                        for half in range(2):
                            for dt_ in range(ET // 2):
                                nc.tensor.matmul(
                                    psl[:, half:half + 1, :],
                                    gls[p][:, 2 * dt_:2 * dt_ + 2,
                                           ob * P:(ob + 1) * P],
                                    otb[:, 2 * dt_:2 * dt_ + 2, half:half + 1, :],
                                    start=(dt_ == 0),
                                    stop=(dt_ == ET // 2 - 1),
                                    perf_mode=DR,
                                )
                        res2 = respool.tile([P, 2, MC], bf16, tag="res",
                                            name="res2")
                        nc.vector.tensor_tensor(
                            res2[:], psl[:], rb2[:], mybir.AluOpType.mult)
                        for half in range(2):
                            mc = 2 * mc2 + half
                            nc.sync.dma_start(
                                rs_in[p][mc][ob * P:(ob + 1) * P, :],
                                res2[:, half, :],
                            )
                    units.append(u)

                def u_rs():
                    for half in range(2):
                        mc = 2 * mc2 + half
                        if not single and not no_cc:
                            nc.gpsimd.collective_compute(
                                "ReduceScatter",
                                mybir.AluOpType.add,
                                ins=[rs_in[p][mc].opt()],
                                outs=[rs_out[p][mc].opt()],
                                replica_groups=[list(range(NCORES))],
                            )
                        po_queue.append(po_chunk(p, mc))
                units.append(u_rs)
                return units

            # ---- 5-stage software pipeline over s = (pair, mc2) ----
            # stage s emits: scores(s+1) fills interleaved with PV/cs(s) and
            # linear(s-1); exp(s+1) runs on ACT behind the fills.
            def interleave(score_us, other_us):
                # spread other units among the score fills, keeping the
                # first out-unit a couple of slots in (exp drain lag)
                ns, no = len(score_us), len(other_us)
                oi = 0
                for i, su in enumerate(score_us):
                    su()
                    while oi * ns < (i + 1) * no:
                        other_us[oi]()
                        oi += 1
                while oi < no:
                    other_us[oi]()
                    oi += 1

            STAGES = [(0, 0), (0, 1), (1, 0), (1, 1)]

            # prologue: A0 + kb0, then scores(0,0) interleaved with A1+kb1
            for u in a_units(0):
                u()
            kb_unit(0)()
            e2_cur, su_cur = scores_units(0, 0)
            p1_feed = a_units(1) + [kb_unit(1)]
            interleave(su_cur, p1_feed)

            lin_prev = None  # linear units deferred from the previous stage
            for si, (p, mc2) in enumerate(STAGES):
                # out-units for THIS stage's e2 + linear for the previous;
                # the last stage has no exp stream, so its ot evacs use ACT
                otb_rb, pvcs_us = pv_cs_units(
                    p, mc2, e2_cur,
                    ot_eng="dve")
                other = []
                if lin_prev:
                    other += lin_prev
                other += pvcs_us
                if si + 1 < len(STAGES):
                    pn, mc2n = STAGES[si + 1]
                    e2_cur, su_next = scores_units(pn, mc2n)
                else:
                    su_next = []
                # flush old epilogue chunks (their RS is long done)
                while len(po_queue) > 2:
                    po_queue.pop(0)()
                if su_next:
                    interleave(su_next, other)
                else:
                    for u in other:
                        u()
                lin_prev = linear_units(p, mc2, otb_rb)

            for u in lin_prev:
                u()
            while po_queue:
                po_queue.pop(0)()

    nc.compile()
    return nc


_NC_CACHE = {}


def _get_nc():
    if "nc" not in _NC_CACHE:
        _NC_CACHE["nc"] = _build()
    return _NC_CACHE["nc"]


def _make_in_maps(inputs):
    f = np.float32
    e4 = ml_dtypes.float8_e4m3

    def c_(x, dt):
        return np.ascontiguousarray(x).astype(dt)

    img = np.asarray(inputs["image_input"], f)
    meta = np.asarray(inputs["metadata_input"], f)
    xt_i = c_(img.T, e4)
    xt_m = c_(meta.T, e4)
    xn_i = c_(img, e4)
    xn_m = c_(meta, e4)

    Wl_i = np.asarray(inputs["W_lin_i"], f)
    Wl_m = np.asarray(inputs["W_lin_m"], f)
    bv_i = np.asarray(inputs["bv_i"], f)
    bv_m = np.asarray(inputs["bv_m"], f)
    # v-bias contribution after softmax normalization: sum_h Wl_h bv_h
    bvterm_i = sum(Wl_i[:, h * D:(h + 1) * D] @ bv_i[h] for h in range(H))
    bvterm_m = sum(Wl_m[:, h * D:(h + 1) * D] @ bv_m[h] for h in range(H))
    add_i = (np.asarray(inputs["b_lin_i"], f) + bvterm_i)
    add_m = (np.asarray(inputs["b_lin_m"], f) + bvterm_m)

    in_maps = []
    for c in range(NCORES):
        sl = slice(64 * c, 64 * (c + 1))
        m = {
            "xt_i": xt_i, "xt_m": xt_m, "xn_i": xn_i, "xn_m": xn_m,
            "resid_i": c_(img[:, sl].T + add_i[sl][:, None], f),
            "resid_m": c_(meta[:, sl].T + add_m[sl][:, None], f),
        }
        for p, (Wq, bq_, Wk, Wv, Wl) in enumerate([
            (inputs["Wq_m"], inputs["bq_m"], inputs["Wk_i"],
             inputs["Wv_i"], Wl_i),
            (inputs["Wq_i"], inputs["bq_i"], inputs["Wk_m"],
             inputs["Wv_m"], Wl_m),
        ]):
            Wqc = np.asarray(Wq, f)[c]
            Wkc = np.asarray(Wk, f)[c]
            Wvc = np.asarray(Wv, f)[c]
            bqc = np.asarray(bq_, f)[c]
            Wlc = Wl[:, D * c:D * (c + 1)]
            m[f"g{p}"] = c_(Wqc.T @ Wkc * WSG, e4)
            m[f"gb{p}"] = c_((Wkc.T @ bqc * WSG)[:, None], e4)
            m[f"gl{p}"] = c_(Wvc.T @ Wlc.T * WSL, e4)
        in_maps.append(m)
    return in_maps


def _assemble(results):
    out_iT = np.concatenate([results[c]["out_i"] for c in range(NCORES)], axis=0)
    out_mT = np.concatenate([results[c]["out_m"] for c in range(NCORES)], axis=0)
    return np.concatenate([out_iT.T, out_mT.T], axis=1).astype(np.float32)


def kernel(**inputs):
    nc = _get_nc()
    in_maps = _make_in_maps(inputs)
    res = run_bass_kernel_spmd(nc, in_maps, list(range(NCORES)))
    return _assemble(res.results)


if __name__ == "__main__":
    _get_nc()
    print("build ok")
